# revision 4
# baseline (speedup 1.0000x reference)
"""ChebConv (K=3) GNN message-passing kernel for 8 Trainium2 NeuronCores.

Strategy (graph/data parallel, per sharding hint):
  - Nodes padded to 100352 and sharded 12544/core (98 tiles of 128).
  - Edges sharded by destination core; within a core grouped by
    (dst tile, src window of 32768 rows) for int16 dma_gather indices.
  - spmm(h) = segment_sum(-norm * h[src], dst) is computed as:
      * scale h rows by dis (per-node) -> H' (gather source, replicated
        via AllGather across the 8 cores),
      * bulk-gather H'[src] rows with gpsimd dma_gather (512B rows),
      * one-hot matrices (dst_local == iota) x gathered rows on the
        TensorEngine accumulate segment sums in PSUM,
      * per-partition scales fold in -dis[dst] and the Chebyshev signs.
  - Chebyshev: Tx0 = x, Tx1 = spmm(x), Tx2 = 2*spmm(Tx1) - x;
    out = Tx0@W0 + Tx1@W1 + Tx2@W2 + bias (PE transposes + matmuls).
"""

import functools
import os
import sys

sys.path.insert(0, "/opt/trn_rl_repo")

import numpy as np

# ---------------------------------------------------------------- constants
N = 100000
E = 1600000
D = 128
P = 128
NCORES = 8
RPC = 12544              # rows per core (NPAD / NCORES)
NPAD = RPC * NCORES      # 100352
TPC = RPC // P           # 98 tiles per core
WIN = 32768              # int16 index window
NWIN = (NPAD + WIN - 1) // WIN  # 4
MAX_IDX = 1024           # dma_gather crashes beyond ~1024 idxs/call


def _ceil(a, b):
    return (a + b - 1) // b


# ---------------------------------------------------------------- host prep
def _preprocess(x, edge_index):
    """Bucket edges by (dst core, dst tile, src window); build per-core
    int16 gather indices, one-hot dst columns, and scale tables."""
    src = edge_index[0].astype(np.int64)
    dst = edge_index[1].astype(np.int64)

    deg = np.bincount(src[src != dst], minlength=N)
    dis = np.zeros(NPAD, np.float32)
    nz = deg > 0
    dis[:N][nz] = (1.0 / np.sqrt(deg[nz])).astype(np.float32)

    keep = src != dst
    src = src[keep]
    dst = dst[keep]

    core = dst // RPC
    tt = (dst % RPC) // P
    g = src // WIN
    dloc = (dst % P).astype(np.int32)

    key = ((core * TPC) + tt) * NWIN + g
    order = np.argsort(key, kind="stable")
    src_s = src[order]
    dloc_s = dloc[order]
    nkey = NCORES * TPC * NWIN
    counts = np.bincount(key, minlength=nkey).reshape(NCORES, TPC, NWIN)
    starts = np.concatenate([[0], np.cumsum(counts.reshape(-1))])[:-1].reshape(
        NCORES, TPC, NWIN
    )

    L = counts.max(axis=0)  # [TPC, NWIN] num_idxs per gather call site
    assert L.max() <= MAX_IDX, f"gather segment too large: {L.max()}"
    nb = _ceil(L, P) * (L > 0)  # blocks per (t, g)
    B_t = nb.sum(axis=1)  # [TPC]
    assert B_t.min() >= 1
    BMAX = int(B_t.max())

    # column offsets of idx segments (in 16-idx columns) and block columns
    cols = _ceil(L, 16)
    coff = np.zeros((TPC, NWIN), np.int64)
    boff = np.zeros((TPC, NWIN), np.int64)
    ci = bi = 0
    for t in range(TPC):
        for w in range(NWIN):
            coff[t, w] = ci
            boff[t, w] = bi
            ci += cols[t, w]
            bi += nb[t, w]
    IWT = int(ci)
    BT = int(bi)

    idx16 = np.zeros((NCORES, P, IWT), np.int16)
    dstc = np.full((NCORES, P, BT), -1.0, np.float32)
    for c in range(NCORES):
        for t in range(TPC):
            for w in range(NWIN):
                l = int(L[t, w])
                if l == 0:
                    continue
                n = int(counts[c, t, w])
                s0 = int(starts[c, t, w])
                ncols = int(cols[t, w])
                a = np.zeros(ncols * 16, np.int16)
                a[:n] = (src_s[s0 : s0 + n] - w * WIN).astype(np.int16)
                wrapped = a.reshape(ncols, 16).T  # [16, ncols]
                idx16[c, :, coff[t, w] : coff[t, w] + ncols] = np.tile(
                    wrapped, (8, 1)
                )
                nblk = int(nb[t, w])
                dl = np.full(nblk * P, -1.0, np.float32)
                dl[:n] = dloc_s[s0 : s0 + n]
                dstc[c, :, boff[t, w] : boff[t, w] + nblk] = dl.reshape(nblk, P).T

    # scale tables [NCORES, P, TPC]
    dl = dis.reshape(NCORES, TPC, P).transpose(0, 2, 1)  # [c, p, t]
    scales = np.stack([dl, -dl, -dl * dl, -2.0 * dl], axis=1).astype(
        np.float32
    )  # [c, 4, p, t]

    tables = {
        "L": L,
        "nb": nb,
        "B_t": B_t,
        "BMAX": BMAX,
        "coff": coff,
        "boff": boff,
        "IWT": IWT,
        "BT": BT,
    }
    return idx16, dstc, scales, tables


# ---------------------------------------------------------------- device IR
def _build_nc(table_key):
    """Build the Bass module. table_key is a hashable encoding of the gather
    tables (L, nb, coff, boff per (tile, window))."""
    import concourse.mybir as mybir
    import concourse.tile as tile
    from concourse import bacc
    from concourse.masks import make_identity

    (L_f, nb_f, coff_f, boff_f, IWT, BT, BMAX) = table_key
    L = np.array(L_f, np.int64).reshape(TPC, NWIN)
    nb = np.array(nb_f, np.int64).reshape(TPC, NWIN)
    coff = np.array(coff_f, np.int64).reshape(TPC, NWIN)
    boff = np.array(boff_f, np.int64).reshape(TPC, NWIN)
    B_t = nb.sum(axis=1)

    f32 = mybir.dt.float32
    nc = bacc.Bacc("TRN2", num_devices=NCORES, num_swdge_queues=4)

    xc = nc.dram_tensor("xc", [RPC, D], f32, kind="ExternalInput")
    idx16 = nc.dram_tensor("idx16", [P, IWT], mybir.dt.int16, kind="ExternalInput")
    dstc = nc.dram_tensor("dstc", [P, BT], f32, kind="ExternalInput")
    scl = nc.dram_tensor("scl", [4, P, TPC], f32, kind="ExternalInput")
    w0 = nc.dram_tensor("w0", [D, D], f32, kind="ExternalInput")
    w1 = nc.dram_tensor("w1", [D, D], f32, kind="ExternalInput")
    w2 = nc.dram_tensor("w2", [D, D], f32, kind="ExternalInput")
    biast = nc.dram_tensor("biast", [P, D], f32, kind="ExternalInput")
    outc = nc.dram_tensor("outc", [RPC, D], f32, kind="ExternalOutput")

    h_own = nc.dram_tensor("h_own", [RPC, D], f32)
    h1_own = nc.dram_tensor("h1_own", [RPC, D], f32)
    h0_full = nc.dram_tensor("h0_full", [NPAD, D], f32, addr_space="Shared")
    h1_full = nc.dram_tensor("h1_full", [NPAD, D], f32, addr_space="Shared")
    tx1_dram = nc.dram_tensor("tx1_dram", [RPC, D], f32)

    rg = [list(range(NCORES))]

    with tile.TileContext(nc, num_cores=NCORES) as tc:
        with (
            tc.tile_pool(name="const", bufs=1) as cp,
            tc.tile_pool(name="gp", bufs=2) as gp,
            tc.tile_pool(name="sb", bufs=3) as sb,
            tc.tile_pool(name="ps", bufs=2, space="PSUM") as ps,
        ):
            identity = cp.tile([P, P], f32)
            make_identity(nc, identity)
            iota = cp.tile([P, P], f32)
            nc.gpsimd.iota(
                iota,
                pattern=[[1, P]],
                base=0,
                channel_multiplier=0,
                allow_small_or_imprecise_dtypes=True,
            )
            w0t = cp.tile([D, D], f32)
            nc.sync.dma_start(w0t, w0[:, :])
            w1t = cp.tile([D, D], f32)
            nc.sync.dma_start(w1t, w1[:, :])
            w2t = cp.tile([D, D], f32)
            nc.sync.dma_start(w2t, w2[:, :])
            bt = cp.tile([P, D], f32)
            nc.sync.dma_start(bt, biast[:, :])
            sclt = []
            for k in range(4):
                sck = cp.tile([P, TPC], f32, name=f"sc{k}")
                nc.sync.dma_start(sck, scl[k])
                sclt.append(sck)
            idxt = cp.tile([P, IWT], mybir.dt.int16)
            nc.sync.dma_start(idxt, idx16[:, :])
            dstt = cp.tile([P, BT], f32)
            nc.sync.dma_start(dstt, dstc[:, :])

            # ---------------- phase 0: h_own = dis * x
            for t in range(TPC):
                xt = sb.tile([P, D], f32, tag="xt")
                nc.sync.dma_start(xt, xc[t * P : (t + 1) * P, :])
                h0 = sb.tile([P, D], f32, tag="h0")
                nc.scalar.activation(
                    h0, xt, mybir.ActivationFunctionType.Copy,
                    scale=sclt[0][:, t : t + 1],
                )
                nc.sync.dma_start(h_own[t * P : (t + 1) * P, :], h0)

            nc.gpsimd.collective_compute(
                "AllGather", mybir.AluOpType.bypass, replica_groups=rg,
                ins=[h_own.ap()], outs=[h0_full.ap()],
            )

            def spmm_tile(t, h_full, first):
                """Gather + segment-sum for tile t from h_full; returns PSUM
                accumulator tile (raw segment sums, one-hot +1 weights)."""
                bt_ = int(B_t[t])
                g = gp.tile([P, BMAX, D], f32, tag="g")
                nc.vector.memset(g[:, :bt_, :], 0.0)
                for w in range(NWIN):
                    l = int(L[t, w])
                    if l == 0:
                        continue
                    nblk = int(nb[t, w])
                    soff = int((boff[t, w] - boff[t, 0]))
                    wbase = w * WIN
                    wrows = min(WIN, NPAD - wbase)
                    nc.gpsimd.dma_gather(
                        g[:, soff : soff + nblk, :],
                        h_full[wbase : wbase + wrows, :],
                        idxt[:, coff[t, w] : coff[t, w] + int(_ceil(l, 16))],
                        l,
                        l,
                        D,
                        queue_num=w,
                    )
                pacc = ps.tile([P, D], f32, tag="pacc")
                for b in range(bt_):
                    m = sb.tile([P, P], f32, tag="m")
                    nc.vector.tensor_tensor(
                        out=m,
                        in0=dstt[:, boff[t, 0] + b : boff[t, 0] + b + 1].to_broadcast(
                            [P, P]
                        ),
                        in1=iota,
                        op=mybir.AluOpType.is_equal,
                    )
                    nc.tensor.matmul(
                        pacc, lhsT=m, rhs=g[:, b, :],
                        start=(b == 0), stop=(b == bt_ - 1),
                    )
                return pacc

            # ---------------- phase 1: Tx1 = spmm(x); H1' = dis*Tx1
            for t in range(TPC):
                pacc = spmm_tile(t, h0_full, first=True)
                tx1 = sb.tile([P, D], f32, tag="tx1")
                nc.scalar.activation(
                    tx1, pacc, mybir.ActivationFunctionType.Copy,
                    scale=sclt[1][:, t : t + 1],
                )
                h1 = sb.tile([P, D], f32, tag="h1")
                nc.scalar.activation(
                    h1, pacc, mybir.ActivationFunctionType.Copy,
                    scale=sclt[2][:, t : t + 1],
                )
                nc.sync.dma_start(tx1_dram[t * P : (t + 1) * P, :], tx1)
                nc.sync.dma_start(h1_own[t * P : (t + 1) * P, :], h1)

            nc.gpsimd.collective_compute(
                "AllGather", mybir.AluOpType.bypass, replica_groups=rg,
                ins=[h1_own.ap()], outs=[h1_full.ap()],
            )

            # ---------------- phase 2: Tx2 = -2dis*P2 - x; out = sum Txk@Wk
            for t in range(TPC):
                pacc = spmm_tile(t, h1_full, first=False)
                xt = sb.tile([P, D], f32, tag="xt2")
                nc.sync.dma_start(xt, xc[t * P : (t + 1) * P, :])
                tx2 = sb.tile([P, D], f32, tag="tx2")
                nc.scalar.activation(
                    tx2, pacc, mybir.ActivationFunctionType.Copy,
                    scale=sclt[3][:, t : t + 1],
                )
                nc.vector.tensor_tensor(
                    out=tx2, in0=tx2, in1=xt, op=mybir.AluOpType.subtract
                )
                tx1 = sb.tile([P, D], f32, tag="tx1b")
                nc.sync.dma_start(tx1, tx1_dram[t * P : (t + 1) * P, :])

                outp = ps.tile([P, D], f32, tag="outp")
                for k, (mat, wk) in enumerate(((xt, w0t), (tx1, w1t), (tx2, w2t))):
                    trp = ps.tile([P, P], f32, tag="trp")
                    nc.tensor.transpose(trp, mat, identity)
                    trs = sb.tile([P, P], f32, tag="trs")
                    nc.scalar.copy(trs, trp)
                    nc.tensor.matmul(
                        outp, lhsT=trs, rhs=wk, start=(k == 0), stop=(k == 2)
                    )
                osb = sb.tile([P, D], f32, tag="osb")
                nc.vector.tensor_add(osb, outp, bt)
                nc.sync.dma_start(outc[t * P : (t + 1) * P, :], osb)

    nc.finalize()
    return nc


@functools.lru_cache(maxsize=2)
def _build_cached(table_key):
    return _build_nc(table_key)


def _table_key(tables):
    return (
        tuple(tables["L"].reshape(-1).tolist()),
        tuple(tables["nb"].reshape(-1).tolist()),
        tuple(tables["coff"].reshape(-1).tolist()),
        tuple(tables["boff"].reshape(-1).tolist()),
        tables["IWT"],
        tables["BT"],
        tables["BMAX"],
    )


_HOOK = [False]


def _maybe_install_ntff_hook():
    """Register the axon NTFF profiling hook (skipped silently if
    unavailable). Only needed when KERNEL_TRACE=1."""
    if _HOOK[0]:
        return
    _HOOK[0] = True
    try:
        import ctypes
        import types

        so_path = "/opt/axon/libaxon_pjrt.so"
        if "antenv.axon_hooks" in sys.modules or not os.path.exists(so_path):
            return
        mod = types.ModuleType("antenv.axon_hooks")
        mod._hook = None
        mod.set_axon_ntff_profile_hook = lambda h: setattr(mod, "_hook", h)
        mod.get_axon_ntff_profile_hook = lambda: mod._hook
        sys.modules["antenv.axon_hooks"] = mod
        import antenv

        antenv.axon_hooks = mod
        if "/root/.axon_site" not in sys.path:
            sys.path.insert(0, "/root/.axon_site")
        from trn_agent_boot.trn_boot import _ntff_profile_via_ctypes

        mod._hook = _ntff_profile_via_ctypes(so_path)
    except Exception:
        pass


def kernel(x, edge_index, weight, bias):
    from concourse.bass_utils import run_bass_kernel_spmd

    trace = bool(int(os.environ.get("KERNEL_TRACE", "0")))
    if trace:
        _maybe_install_ntff_hook()

    x = np.asarray(x, np.float32)
    edge_index = np.asarray(edge_index)
    weight = np.asarray(weight, np.float32)
    bias = np.asarray(bias, np.float32)

    idx16, dstc, scales, tables = _preprocess(x, edge_index)
    nc = _build_cached(_table_key(tables))

    xpad = np.zeros((NPAD, D), np.float32)
    xpad[:N] = x
    bias_tile = np.tile(bias[None, :], (P, 1)).astype(np.float32)

    in_maps = []
    for c in range(NCORES):
        in_maps.append(
            {
                "xc": np.ascontiguousarray(xpad[c * RPC : (c + 1) * RPC]),
                "idx16": np.ascontiguousarray(idx16[c]),
                "dstc": np.ascontiguousarray(dstc[c]),
                "scl": np.ascontiguousarray(scales[c]),
                "w0": np.ascontiguousarray(weight[0]),
                "w1": np.ascontiguousarray(weight[1]),
                "w2": np.ascontiguousarray(weight[2]),
                "biast": bias_tile,
            }
        )

    res = run_bass_kernel_spmd(
        nc, in_maps, core_ids=list(range(NCORES)), trace=trace
    )
    if trace and res.exec_time_ns:
        print(f"HW exec time: {res.exec_time_ns} ns", flush=True)

    out = np.concatenate([res.results[c]["outc"] for c in range(NCORES)], axis=0)
    return out[:N]


# revision 5
# speedup vs baseline: 1.3075x; 1.3075x over previous
"""ChebConv (K=3) GNN message-passing kernel for 8 Trainium2 NeuronCores.

Strategy (graph/data parallel, per sharding hint):
  - Nodes padded to 100352 and sharded 12544/core (98 tiles of 128).
  - Edges sharded by destination core; within a core grouped by
    (dst tile, src window of 32768 rows) for int16 dma_gather indices.
  - spmm(h) = segment_sum(-norm * h[src], dst) is computed as:
      * scale h rows by dis (per-node) -> H' (gather source, replicated
        via AllGather across the 8 cores),
      * bulk-gather H'[src] rows with gpsimd dma_gather (512B rows),
      * one-hot matrices (dst_local == iota) x gathered rows on the
        TensorEngine accumulate segment sums in PSUM,
      * per-partition scales fold in -dis[dst] and the Chebyshev signs.
  - Chebyshev: Tx0 = x, Tx1 = spmm(x), Tx2 = 2*spmm(Tx1) - x;
    out = Tx0@W0 + Tx1@W1 + Tx2@W2 + bias (PE transposes + matmuls).
"""

import functools
import os
import sys

sys.path.insert(0, "/opt/trn_rl_repo")

import numpy as np

# ---------------------------------------------------------------- constants
N = 100000
E = 1600000
D = 128
P = 128
NCORES = 8
RPC = 12544              # rows per core (NPAD / NCORES)
NPAD = RPC * NCORES      # 100352
TPC = RPC // P           # 98 tiles per core
WIN = 32768              # int16 index window
NWIN = (NPAD + WIN - 1) // WIN  # 4
MAX_IDX = 1024           # dma_gather crashes beyond ~1024 idxs/call


def _ceil(a, b):
    return (a + b - 1) // b


# ---------------------------------------------------------------- host prep
def _preprocess(x, edge_index):
    """Bucket edges by (dst core, dst tile, src window); build per-core
    int16 gather indices, one-hot dst columns, and scale tables."""
    src = edge_index[0].astype(np.int64)
    dst = edge_index[1].astype(np.int64)

    deg = np.bincount(src[src != dst], minlength=N)
    dis = np.zeros(NPAD, np.float32)
    nz = deg > 0
    dis[:N][nz] = (1.0 / np.sqrt(deg[nz])).astype(np.float32)

    keep = src != dst
    src = src[keep]
    dst = dst[keep]

    core = dst // RPC
    tt = (dst % RPC) // P
    g = src // WIN
    dloc = (dst % P).astype(np.int32)

    key = ((core * TPC) + tt) * NWIN + g
    order = np.argsort(key, kind="stable")
    src_s = src[order]
    dloc_s = dloc[order]
    nkey = NCORES * TPC * NWIN
    counts = np.bincount(key, minlength=nkey).reshape(NCORES, TPC, NWIN)
    starts = np.concatenate([[0], np.cumsum(counts.reshape(-1))])[:-1].reshape(
        NCORES, TPC, NWIN
    )

    L = counts.max(axis=0)  # [TPC, NWIN] num_idxs per gather call site
    assert L.max() <= MAX_IDX, f"gather segment too large: {L.max()}"
    nb = _ceil(L, P) * (L > 0)  # blocks per (t, g)
    B_t = nb.sum(axis=1)  # [TPC]
    assert B_t.min() >= 1
    BMAX = int(B_t.max())

    # column offsets of idx segments (in 16-idx columns) and block columns
    cols = _ceil(L, 16)
    coff = np.zeros((TPC, NWIN), np.int64)
    boff = np.zeros((TPC, NWIN), np.int64)
    ci = bi = 0
    for t in range(TPC):
        for w in range(NWIN):
            coff[t, w] = ci
            boff[t, w] = bi
            ci += cols[t, w]
            bi += nb[t, w]
    IWT = int(ci)
    BT = int(bi)

    idx16 = np.zeros((NCORES, P, IWT), np.int16)
    dstc = np.full((NCORES, P, BT), -1.0, np.float32)
    for c in range(NCORES):
        for t in range(TPC):
            for w in range(NWIN):
                l = int(L[t, w])
                if l == 0:
                    continue
                n = int(counts[c, t, w])
                s0 = int(starts[c, t, w])
                ncols = int(cols[t, w])
                a = np.zeros(ncols * 16, np.int16)
                a[:n] = (src_s[s0 : s0 + n] - w * WIN).astype(np.int16)
                wrapped = a.reshape(ncols, 16).T  # [16, ncols]
                idx16[c, :, coff[t, w] : coff[t, w] + ncols] = np.tile(
                    wrapped, (8, 1)
                )
                nblk = int(nb[t, w])
                dl = np.full(nblk * P, -1.0, np.float32)
                dl[:n] = dloc_s[s0 : s0 + n]
                dstc[c, :, boff[t, w] : boff[t, w] + nblk] = dl.reshape(nblk, P).T

    # scale tables [NCORES, P, TPC]
    dl = dis.reshape(NCORES, TPC, P).transpose(0, 2, 1)  # [c, p, t]
    scales = np.stack([dl, -dl, -dl * dl, -2.0 * dl], axis=1).astype(
        np.float32
    )  # [c, 4, p, t]

    tables = {
        "L": L,
        "nb": nb,
        "B_t": B_t,
        "BMAX": BMAX,
        "coff": coff,
        "boff": boff,
        "IWT": IWT,
        "BT": BT,
    }
    return idx16, dstc, scales, tables


# ---------------------------------------------------------------- device IR
def _build_nc(table_key):
    """Build the Bass module. table_key is a hashable encoding of the gather
    tables (L, nb, coff, boff per (tile, window))."""
    import concourse.mybir as mybir
    import concourse.tile as tile
    from concourse import bacc
    from concourse.masks import make_identity

    (L_f, nb_f, coff_f, boff_f, IWT, BT, BMAX) = table_key
    L = np.array(L_f, np.int64).reshape(TPC, NWIN)
    nb = np.array(nb_f, np.int64).reshape(TPC, NWIN)
    coff = np.array(coff_f, np.int64).reshape(TPC, NWIN)
    boff = np.array(boff_f, np.int64).reshape(TPC, NWIN)
    B_t = nb.sum(axis=1)

    f32 = mybir.dt.float32
    nc = bacc.Bacc("TRN2", num_devices=NCORES, num_swdge_queues=4)

    xc = nc.dram_tensor("xc", [RPC, D], f32, kind="ExternalInput")
    idx16 = nc.dram_tensor("idx16", [P, IWT], mybir.dt.int16, kind="ExternalInput")
    dstc = nc.dram_tensor("dstc", [P, BT], f32, kind="ExternalInput")
    scl = nc.dram_tensor("scl", [4, P, TPC], f32, kind="ExternalInput")
    w0 = nc.dram_tensor("w0", [D, D], f32, kind="ExternalInput")
    w1 = nc.dram_tensor("w1", [D, D], f32, kind="ExternalInput")
    w2 = nc.dram_tensor("w2", [D, D], f32, kind="ExternalInput")
    biast = nc.dram_tensor("biast", [P, D], f32, kind="ExternalInput")
    outc = nc.dram_tensor("outc", [RPC, D], f32, kind="ExternalOutput")

    h_own = nc.dram_tensor("h_own", [RPC, D], f32)
    h1_own = nc.dram_tensor("h1_own", [RPC, D], f32)
    h0_full = nc.dram_tensor("h0_full", [NPAD, D], f32, addr_space="Shared")
    h1_full = nc.dram_tensor("h1_full", [NPAD, D], f32, addr_space="Shared")
    tx1_dram = nc.dram_tensor("tx1_dram", [RPC, D], f32)

    rg = [list(range(NCORES))]

    with tile.TileContext(nc, num_cores=NCORES) as tc:
        with (
            tc.tile_pool(name="const", bufs=1) as cp,
            tc.tile_pool(name="sb", bufs=3) as sb,
            tc.tile_pool(name="ps", bufs=2, space="PSUM") as ps,
        ):
            identity = cp.tile([P, P], f32)
            make_identity(nc, identity)
            iota = cp.tile([P, P], f32)
            nc.gpsimd.iota(
                iota,
                pattern=[[1, P]],
                base=0,
                channel_multiplier=0,
                allow_small_or_imprecise_dtypes=True,
            )
            w0t = cp.tile([D, D], f32)
            nc.sync.dma_start(w0t, w0[:, :])
            w1t = cp.tile([D, D], f32)
            nc.sync.dma_start(w1t, w1[:, :])
            w2t = cp.tile([D, D], f32)
            nc.sync.dma_start(w2t, w2[:, :])
            bt = cp.tile([P, D], f32)
            nc.sync.dma_start(bt, biast[:, :])
            sclt = []
            for k in range(4):
                sck = cp.tile([P, TPC], f32, name=f"sc{k}")
                nc.sync.dma_start(sck, scl[k])
                sclt.append(sck)
            idxt = cp.tile([P, IWT], mybir.dt.int16)
            nc.sync.dma_start(idxt, idx16[:, :])
            GDEPTH = 3
            gbig = cp.tile([P, GDEPTH * BMAX, D], f32)
            nc.vector.memset(gbig[:, :, :], 0.0)
            dstt = cp.tile([P, BT], f32)
            nc.sync.dma_start(dstt, dstc[:, :])

            # ---------------- phase 0: h_own = dis * x
            for t in range(TPC):
                xt = sb.tile([P, D], f32, tag="xt")
                nc.sync.dma_start(xt, xc[t * P : (t + 1) * P, :])
                h0 = sb.tile([P, D], f32, tag="h0")
                nc.scalar.activation(
                    h0, xt, mybir.ActivationFunctionType.Copy,
                    scale=sclt[0][:, t : t + 1],
                )
                nc.sync.dma_start(h_own[t * P : (t + 1) * P, :], h0)

            nc.gpsimd.collective_compute(
                "AllGather", mybir.AluOpType.bypass, replica_groups=rg,
                ins=[h_own.ap()], outs=[h0_full.ap()],
            )

            def spmm_tile(t, h_full, first):
                """Gather + segment-sum for tile t from h_full; returns PSUM
                accumulator tile (raw segment sums, one-hot +1 weights)."""
                bt_ = int(B_t[t])
                g = gbig[:, (t % GDEPTH) * BMAX : (t % GDEPTH) * BMAX + BMAX, :]
                for w in range(NWIN):
                    l = int(L[t, w])
                    if l == 0:
                        continue
                    nblk = int(nb[t, w])
                    soff = int((boff[t, w] - boff[t, 0]))
                    wbase = w * WIN
                    wrows = min(WIN, NPAD - wbase)
                    nc.gpsimd.dma_gather(
                        g[:, soff : soff + nblk, :],
                        h_full[wbase : wbase + wrows, :],
                        idxt[:, coff[t, w] : coff[t, w] + int(_ceil(l, 16))],
                        l,
                        l,
                        D,
                        queue_num=w,
                    )
                pacc = ps.tile([P, D], f32, tag="pacc", bufs=3)
                for b in range(bt_):
                    m = sb.tile([P, P], f32, tag="m")
                    nc.vector.tensor_tensor(
                        out=m,
                        in0=dstt[:, boff[t, 0] + b : boff[t, 0] + b + 1].to_broadcast(
                            [P, P]
                        ),
                        in1=iota,
                        op=mybir.AluOpType.is_equal,
                    )
                    nc.tensor.matmul(
                        pacc, lhsT=m, rhs=g[:, b, :],
                        start=(b == 0), stop=(b == bt_ - 1),
                    )
                return pacc

            # ---------------- phase 1: Tx1 = spmm(x); H1' = dis*Tx1
            for t in range(TPC):
                pacc = spmm_tile(t, h0_full, first=True)
                tx1 = sb.tile([P, D], f32, tag="tx1")
                nc.scalar.activation(
                    tx1, pacc, mybir.ActivationFunctionType.Copy,
                    scale=sclt[1][:, t : t + 1],
                )
                h1 = sb.tile([P, D], f32, tag="h1")
                nc.scalar.activation(
                    h1, pacc, mybir.ActivationFunctionType.Copy,
                    scale=sclt[2][:, t : t + 1],
                )
                nc.sync.dma_start(tx1_dram[t * P : (t + 1) * P, :], tx1)
                nc.sync.dma_start(h1_own[t * P : (t + 1) * P, :], h1)

            nc.gpsimd.collective_compute(
                "AllGather", mybir.AluOpType.bypass, replica_groups=rg,
                ins=[h1_own.ap()], outs=[h1_full.ap()],
            )

            # ---------------- phase 2: Tx2 = -2dis*P2 - x; out = sum Txk@Wk
            for t in range(TPC):
                pacc = spmm_tile(t, h1_full, first=False)
                xt = sb.tile([P, D], f32, tag="xt2")
                nc.sync.dma_start(xt, xc[t * P : (t + 1) * P, :])
                tx2 = sb.tile([P, D], f32, tag="tx2")
                nc.scalar.activation(
                    tx2, pacc, mybir.ActivationFunctionType.Copy,
                    scale=sclt[3][:, t : t + 1],
                )
                nc.vector.tensor_tensor(
                    out=tx2, in0=tx2, in1=xt, op=mybir.AluOpType.subtract
                )
                tx1 = sb.tile([P, D], f32, tag="tx1b")
                nc.sync.dma_start(tx1, tx1_dram[t * P : (t + 1) * P, :])

                outp = ps.tile([P, D], f32, tag="outp")
                for k, (mat, wk) in enumerate(((xt, w0t), (tx1, w1t), (tx2, w2t))):
                    trp = ps.tile([P, P], f32, tag="trp")
                    nc.tensor.transpose(trp, mat, identity)
                    trs = sb.tile([P, P], f32, tag="trs")
                    nc.scalar.copy(trs, trp)
                    nc.tensor.matmul(
                        outp, lhsT=trs, rhs=wk, start=(k == 0), stop=(k == 2)
                    )
                osb = sb.tile([P, D], f32, tag="osb")
                nc.vector.tensor_add(osb, outp, bt)
                nc.sync.dma_start(outc[t * P : (t + 1) * P, :], osb)

    nc.finalize()
    return nc


@functools.lru_cache(maxsize=2)
def _build_cached(table_key):
    return _build_nc(table_key)


def _table_key(tables):
    return (
        tuple(tables["L"].reshape(-1).tolist()),
        tuple(tables["nb"].reshape(-1).tolist()),
        tuple(tables["coff"].reshape(-1).tolist()),
        tuple(tables["boff"].reshape(-1).tolist()),
        tables["IWT"],
        tables["BT"],
        tables["BMAX"],
    )


_HOOK = [False]


def _maybe_install_ntff_hook():
    """Register the axon NTFF profiling hook (skipped silently if
    unavailable). Only needed when KERNEL_TRACE=1."""
    if _HOOK[0]:
        return
    _HOOK[0] = True
    try:
        import ctypes
        import types

        so_path = "/opt/axon/libaxon_pjrt.so"
        if "antenv.axon_hooks" in sys.modules or not os.path.exists(so_path):
            return
        mod = types.ModuleType("antenv.axon_hooks")
        mod._hook = None
        mod.set_axon_ntff_profile_hook = lambda h: setattr(mod, "_hook", h)
        mod.get_axon_ntff_profile_hook = lambda: mod._hook
        sys.modules["antenv.axon_hooks"] = mod
        import antenv

        antenv.axon_hooks = mod
        if "/root/.axon_site" not in sys.path:
            sys.path.insert(0, "/root/.axon_site")
        from trn_agent_boot.trn_boot import _ntff_profile_via_ctypes

        mod._hook = _ntff_profile_via_ctypes(so_path)
    except Exception:
        pass


def kernel(x, edge_index, weight, bias):
    from concourse.bass_utils import run_bass_kernel_spmd

    trace = bool(int(os.environ.get("KERNEL_TRACE", "0")))
    if trace:
        _maybe_install_ntff_hook()

    x = np.asarray(x, np.float32)
    edge_index = np.asarray(edge_index)
    weight = np.asarray(weight, np.float32)
    bias = np.asarray(bias, np.float32)

    idx16, dstc, scales, tables = _preprocess(x, edge_index)
    nc = _build_cached(_table_key(tables))

    xpad = np.zeros((NPAD, D), np.float32)
    xpad[:N] = x
    bias_tile = np.tile(bias[None, :], (P, 1)).astype(np.float32)

    in_maps = []
    for c in range(NCORES):
        in_maps.append(
            {
                "xc": np.ascontiguousarray(xpad[c * RPC : (c + 1) * RPC]),
                "idx16": np.ascontiguousarray(idx16[c]),
                "dstc": np.ascontiguousarray(dstc[c]),
                "scl": np.ascontiguousarray(scales[c]),
                "w0": np.ascontiguousarray(weight[0]),
                "w1": np.ascontiguousarray(weight[1]),
                "w2": np.ascontiguousarray(weight[2]),
                "biast": bias_tile,
            }
        )

    res = run_bass_kernel_spmd(
        nc, in_maps, core_ids=list(range(NCORES)), trace=trace
    )
    if trace and res.exec_time_ns:
        print(f"HW exec time: {res.exec_time_ns} ns", flush=True)

    out = np.concatenate([res.results[c]["outc"] for c in range(NCORES)], axis=0)
    return out[:N]
